# revision 1
# baseline (speedup 1.0000x reference)
"""Trainium2 Bass kernel for nn_Complex_Fully_Connected_Linear_Discriminator_LPF.

Strategy (8 NeuronCores):
  - Stage 1 (input projection): batch-sharded (32 samples/core). One folded GEMM
    X' @ Wbig with Wbig = [[Ur^T, Ui^T], [-Ui^T, Ur^T]] produces the per-step scan
    constants C_r, C_i directly (C_r = xr@Ur^T - xi@Ui^T etc).
  - Stage 2 (recurrent scan, 64 steps): batch-sharded. State kept transposed
    (feature-partitioned stationary), step GEMM uses PE column-tiling to run the
    [hrT|hiT]xWr^T and [-hiT|hrT]xWi^T streams concurrently; the r/i combining
    then becomes a single DVE add of psum[0:64]+psum[64:128]. C is injected via
    identity-matmul accumulation into PSUM. State transposed back each step on PE.
  - Stage 3 (MLP l1-l3): feature-sharded (each core owns 384 output features of
    each layer), full batch, with AllGather of activations between layers.
    Activations kept transposed [feat, sample-stack] so no transposes are needed.
  - l5: per-core partial dot products, AllGather + on-device rank-sum + lrelu.
All matmuls in bf16 (fp32 accumulate).
"""

import numpy as np
import ml_dtypes

B, T = 256, 64
H = 768          # hidden (=N_IN/2)
NIN = 1536
W2 = 3072
NC = 8
BS = B // NC     # 32 samples per core
FS = W2 // NC    # 384 output features per core in MLP
BF = ml_dtypes.bfloat16

_BUILD_CACHE = {}


def _build_program():
    import concourse.bacc as bacc
    import concourse.mybir as mybir
    import concourse.tile as tile

    f32 = mybir.dt.float32
    bf16 = mybir.dt.bfloat16
    PRELU = mybir.ActivationFunctionType.Prelu

    nc = bacc.Bacc("TRN2", target_bir_lowering=False, debug=False, num_devices=NC)

    # ---- I/O ----
    d_xt = nc.dram_tensor("xt", [NIN, 2048], bf16, kind="ExternalInput").ap()
    d_wbig = nc.dram_tensor("wbig", [NIN, NIN], bf16, kind="ExternalInput").ap()
    d_wrt = nc.dram_tensor("wrt", [H, H], bf16, kind="ExternalInput").ap()
    d_wit = nc.dram_tensor("wit", [H, H], bf16, kind="ExternalInput").ap()
    d_s0t = nc.dram_tensor("s0t", [128, 6, 64], bf16, kind="ExternalInput").ap()
    d_s0nt = nc.dram_tensor("s0nt", [128, 6, 64], bf16, kind="ExternalInput").ap()
    d_cw1 = nc.dram_tensor("cw1", [H, 2 * FS], bf16, kind="ExternalInput").ap()
    d_cw2 = nc.dram_tensor("cw2", [W2, 2 * FS], bf16, kind="ExternalInput").ap()
    d_cw3 = nc.dram_tensor("cw3", [W2, 2 * FS], bf16, kind="ExternalInput").ap()
    d_w5 = nc.dram_tensor("w5", [128, 6], bf16, kind="ExternalInput").ap()
    d_ia = nc.dram_tensor("ia", [128, 32], bf16, kind="ExternalInput").ap()
    d_id64 = nc.dram_tensor("id64", [64, 64], bf16, kind="ExternalInput").ap()
    d_out = nc.dram_tensor("out", [B, 1], f32, kind="ExternalOutput").ap()

    with tile.TileContext(nc) as tc:
        with (
            tc.tile_pool(name="pmain", bufs=1) as pmain,
            tc.tile_pool(name="pstate", bufs=2) as pstate,
            tc.tile_pool(name="pdram", bufs=1, space="DRAM") as pdram,
        ):
            # persistent SBUF tiles
            cr_t = pmain.tile([128, 16, H], bf16, tag="cr")
            ci_t = pmain.tile([128, 16, H], bf16, tag="ci")
            wrt_sb = pmain.tile([128, 6, H], bf16, tag="wrt")
            wit_sb = pmain.tile([128, 6, H], bf16, tag="wit")
            ia_sb = pmain.tile([128, 32], bf16, tag="ia")
            id64_sb = pmain.tile([64, 64], bf16, tag="id64")
            w5_sb = pmain.tile([128, 6], bf16, tag="w5")
            a1_sb = pmain.tile([128, 6, NC, 64], bf16, tag="a1")
            ones8 = pmain.tile([8, 1], f32, tag="ones8")
            g5_sb = pmain.tile([8, B], f32, tag="g5")
            o5_sb = pmain.tile([1, B], f32, tag="o5")

            nc.sync.dma_start(wrt_sb[:], d_wrt.rearrange("(k p) n -> p k n", p=128))
            nc.sync.dma_start(wit_sb[:], d_wit.rearrange("(k p) n -> p k n", p=128))
            nc.sync.dma_start(ia_sb[:], d_ia)
            nc.sync.dma_start(id64_sb[:], d_id64)
            nc.sync.dma_start(w5_sb[:], d_w5)
            nc.gpsimd.memset(ones8[:], 1.0)

            # DRAM bounce buffers for collectives
            b_s = pdram.tile([6, 128, 64], bf16, tag="b_s")
            b_sg = pdram.tile([NC, 6, 128, 64], bf16, tag="b_sg", addr_space="Shared")
            b_xo = pdram.tile([3, 128, NC, 64], bf16, tag="b_xo")
            b_xg1 = pdram.tile([NC, 3, 128, NC, 64], bf16, tag="b_xg1", addr_space="Shared")
            b_xg2 = pdram.tile([NC, 3, 128, NC, 64], bf16, tag="b_xg2", addr_space="Shared")
            b_5 = pdram.tile([1, B], f32, tag="b_5")
            b_5g = pdram.tile([NC, B], f32, tag="b_5g", addr_space="Shared")

            # ---------------- Stage 1: input projection ----------------
            with (
                tc.tile_pool(name="ps1", bufs=1) as ps1,
                tc.tile_pool(name="pxt", bufs=4) as pxt,
                tc.tile_pool(name="pps1", bufs=1, space="PSUM") as pps1,
                tc.tile_pool(name="ppscan", bufs=1, space="PSUM") as ppscan,
            ):
                wbig_sb = ps1.tile([128, 12, NIN], bf16, tag="wbig")
                nc.sync.dma_start(
                    wbig_sb[:], d_wbig.rearrange("(k p) n -> p k n", p=128)
                )
                for m in range(16):
                    pc_r = pps1.tile([128, H], f32, tag="pc_r")
                    pc_i = pps1.tile([128, H], f32, tag="pc_i")
                    for k in range(12):
                        x_t = pxt.tile([128, 128], bf16, tag="x_t")
                        nc.sync.dma_start(
                            x_t[:],
                            d_xt[128 * k : 128 * k + 128, 128 * m : 128 * m + 128],
                        )
                        st = k == 0
                        sp = k == 11
                        nc.tensor.matmul(
                            pc_r[:, 0:512], x_t[:], wbig_sb[:, k, 0:512],
                            start=st, stop=sp,
                        )
                        nc.tensor.matmul(
                            pc_r[:, 512:768], x_t[:], wbig_sb[:, k, 512:768],
                            start=st, stop=sp,
                        )
                        nc.tensor.matmul(
                            pc_i[:, 0:512], x_t[:], wbig_sb[:, k, 768:1280],
                            start=st, stop=sp,
                        )
                        nc.tensor.matmul(
                            pc_i[:, 512:768], x_t[:], wbig_sb[:, k, 1280:1536],
                            start=st, stop=sp,
                        )
                    nc.vector.tensor_copy(cr_t[:, m, :], pc_r[:])
                    nc.scalar.copy(ci_t[:, m, :], pc_i[:])

                # ---------------- Stage 2: recurrent scan ----------------
                stt = pstate.tile([128, 6, 64], bf16, tag="stt")
                snt = pstate.tile([128, 6, 64], bf16, tag="snt")
                nc.sync.dma_start(stt[:], d_s0t)
                nc.sync.dma_start(snt[:], d_s0nt)

                for t in range(T):
                    g = t % 4
                    blk = t // 4
                    ps = ppscan.tile([128, H], f32, tag="ps")
                    for k in range(6):
                        st = k == 0
                        nc.tensor.matmul(
                            ps[0:64, 0:512], stt[:, k, :], wrt_sb[:, k, 0:512],
                            tile_position=(0, 0), start=st, stop=False,
                        )
                        nc.tensor.matmul(
                            ps[64:128, 0:512], snt[:, k, :], wit_sb[:, k, 0:512],
                            tile_position=(0, 64), start=st, stop=(k == 5),
                        )
                        nc.tensor.matmul(
                            ps[0:64, 512:768], stt[:, k, :], wrt_sb[:, k, 512:768],
                            tile_position=(0, 0), start=st, stop=False,
                        )
                        nc.tensor.matmul(
                            ps[64:128, 512:768], snt[:, k, :], wit_sb[:, k, 512:768],
                            tile_position=(0, 64), start=st, stop=(k == 5),
                        )
                    # C injection via identity accumulate (rows 0:32 <- C_r, 32:64 <- C_i)
                    nc.tensor.matmul(
                        ps[0:32, 0:512], ia_sb[32 * g : 32 * g + 32, :],
                        cr_t[32 * g : 32 * g + 32, blk, 0:512],
                        tile_position=(32 * g, 0), start=False, stop=False,
                    )
                    nc.tensor.matmul(
                        ps[0:32, 512:768], ia_sb[32 * g : 32 * g + 32, :],
                        cr_t[32 * g : 32 * g + 32, blk, 512:768],
                        tile_position=(32 * g, 0), start=False, stop=True,
                    )
                    nc.tensor.matmul(
                        ps[32:64, 0:512], ia_sb[32 * g : 32 * g + 32, :],
                        ci_t[32 * g : 32 * g + 32, blk, 0:512],
                        tile_position=(32 * g, 32), start=False, stop=False,
                    )
                    nc.tensor.matmul(
                        ps[32:64, 512:768], ia_sb[32 * g : 32 * g + 32, :],
                        ci_t[32 * g : 32 * g + 32, blk, 512:768],
                        tile_position=(32 * g, 32), start=False, stop=True,
                    )
                    ybot = pstate.tile([64, H], f32, tag="ybot")
                    nc.scalar.copy(ybot[:], ps[64:128, :])
                    s_pre = pstate.tile([64, H], f32, tag="s_pre")
                    nc.vector.tensor_add(s_pre[:], ps[0:64, :], ybot[:])
                    snew = pstate.tile([64, H], bf16, tag="snew")
                    nc.scalar.activation(snew[:], s_pre[:], PRELU, alpha=0.1)
                    psT = ppscan.tile([128, 6, 64], bf16, tag="psT", bufs=2)
                    for k in range(6):
                        nc.tensor.transpose(
                            psT[:, k, :], snew[:, 128 * k : 128 * k + 128], id64_sb[:]
                        )
                    stt = pstate.tile([128, 6, 64], bf16, tag="stt")
                    nc.vector.tensor_copy(stt[:], psT[:])
                    if t < T - 1:
                        snt = pstate.tile([128, 6, 64], bf16, tag="snt")
                        nc.vector.tensor_scalar_mul(snt[:, :, 0:32], psT[:, :, 32:64], -1.0)
                        nc.vector.tensor_copy(snt[:, :, 32:64], psT[:, :, 0:32])

                # ---------------- AllGather scan state ----------------
                nc.sync.dma_start(b_s[:].rearrange("k p u -> p k u"), stt[:])
                nc.gpsimd.collective_compute(
                    "AllGather", mybir.AluOpType.bypass,
                    replica_groups=[list(range(NC))],
                    ins=[b_s.opt()], outs=[b_sg.opt()],
                )
                for k in range(6):
                    nc.sync.dma_start(
                        a1_sb[:, k, :, :],
                        b_sg[:, k, :, :].rearrange("c p u -> p c u"),
                    )

            # ---------------- Stage 3: MLP ----------------
            with (
                tc.tile_pool(name="pmlp", bufs=1) as pmlp,
                tc.tile_pool(name="pwk", bufs=8) as pwk,
                tc.tile_pool(name="pxn", bufs=2) as pxn,
                tc.tile_pool(name="pyb", bufs=6) as pyb,
                tc.tile_pool(name="ppm", bufs=6, space="PSUM") as ppm,
                tc.tile_pool(name="pp5", bufs=1, space="PSUM") as pp5,
            ):
                a_mlp = pmlp.tile([128, 24, NC, 64], bf16, tag="a_mlp")

                def mlp_layer(a_tile, d_cw, kchunks, out_xn):
                    pys = [
                        ppm.tile([128, NC, 64], f32, tag="py", name=f"py{_mb}")
                        for _mb in range(6)
                    ]
                    for k in range(kchunks):
                        wk = pwk.tile([128, 2 * FS], bf16, tag="wk")
                        nc.sync.dma_start(
                            wk[:], d_cw[128 * k : 128 * k + 128, :]
                        )
                        for mb in range(6):
                            nc.tensor.matmul(
                                pys[mb][:],
                                wk[:, 128 * mb : 128 * mb + 128],
                                a_tile[:, k, :, :],
                                start=(k == 0), stop=(k == kchunks - 1),
                            )
                    ys = []
                    for mb in range(6):
                        y = pyb.tile([128, NC, 64], bf16, tag="y")
                        nc.scalar.activation(y[:], pys[mb][:], PRELU, alpha=0.1)
                        ys.append(y)
                    for mb in range(3):
                        # xrn^T (r-cols): yrr - yii ; xin^T (i-cols): yir + yri
                        nc.vector.tensor_sub(
                            out_xn[:, mb, :, 0:32],
                            ys[mb][:, :, 0:32], ys[mb + 3][:, :, 32:64],
                        )
                        nc.vector.tensor_add(
                            out_xn[:, mb, :, 32:64],
                            ys[mb][:, :, 32:64], ys[mb + 3][:, :, 0:32],
                        )

                def ag_xn(xn_tile, a_dst, b_gather):
                    nc.sync.dma_start(
                        b_xo[:].rearrange("j p c u -> p j c u"), xn_tile[:]
                    )
                    nc.gpsimd.collective_compute(
                        "AllGather", mybir.AluOpType.bypass,
                        replica_groups=[list(range(NC))],
                        ins=[b_xo.opt()], outs=[b_gather.opt()],
                    )
                    nc.sync.dma_start(
                        a_dst[:].rearrange("p k g u -> p k (g u)"),
                        b_gather[:].rearrange("c j p g u -> p (c j) (g u)"),
                    )

                xn1 = pxn.tile([128, 3, NC, 64], bf16, tag="xn")
                mlp_layer(a1_sb, d_cw1, 6, xn1)
                ag_xn(xn1, a_mlp, b_xg1)
                xn2 = pxn.tile([128, 3, NC, 64], bf16, tag="xn")
                mlp_layer(a_mlp, d_cw2, 24, xn2)
                ag_xn(xn2, a_mlp, b_xg2)
                xl = pxn.tile([128, 3, NC, 64], bf16, tag="xn")
                mlp_layer(a_mlp, d_cw3, 24, xl)

                # ---------------- l5 ----------------
                p5 = pp5.tile([1, NC, 32], f32, tag="p5")
                for j in range(3):
                    nc.tensor.matmul(
                        p5[:], w5_sb[:, j : j + 1], xl[:, j, :, 0:32],
                        start=(j == 0), stop=False,
                    )
                for j in range(3):
                    nc.tensor.matmul(
                        p5[:], w5_sb[:, 3 + j : 4 + j], xl[:, j, :, 32:64],
                        start=False, stop=(j == 2),
                    )
                sp5 = pmlp.tile([1, B], f32, tag="sp5")
                nc.vector.tensor_copy(sp5[:], p5[:].rearrange("p c u -> p (c u)"))
                nc.sync.dma_start(b_5[:], sp5[:])
                nc.gpsimd.collective_compute(
                    "AllGather", mybir.AluOpType.bypass,
                    replica_groups=[list(range(NC))],
                    ins=[b_5.opt()], outs=[b_5g.opt()],
                )
                nc.sync.dma_start(g5_sb[:], b_5g[:])
                p5f = pp5.tile([1, B], f32, tag="p5f")
                nc.tensor.matmul(p5f[:], ones8[:], g5_sb[:], start=True, stop=True)
                nc.scalar.activation(o5_sb[:], p5f[:], PRELU, alpha=0.1)
                nc.sync.dma_start(d_out.rearrange("b one -> one b"), o5_sb[:])

    nc.compile()
    return nc


def _prep_inputs(inputs):
    """Host-side sharding/layout prep. Returns in_maps (list of dicts per core)."""
    f = np.float32
    x = np.asarray(inputs["x"], dtype=f)
    h0r = np.asarray(inputs["h0r"], dtype=f)
    h0i = np.asarray(inputs["h0i"], dtype=f)
    Ur = np.asarray(inputs["Ur_w"], dtype=f)
    Ui = np.asarray(inputs["Ui_w"], dtype=f)
    Wr = np.asarray(inputs["Wr_w"], dtype=f)
    Wi = np.asarray(inputs["Wi_w"], dtype=f)
    l1r = np.asarray(inputs["l1r_w"], dtype=f)
    l1i = np.asarray(inputs["l1i_w"], dtype=f)
    l2r = np.asarray(inputs["l2r_w"], dtype=f)
    l2i = np.asarray(inputs["l2i_w"], dtype=f)
    l3r = np.asarray(inputs["l3r_w"], dtype=f)
    l3i = np.asarray(inputs["l3i_w"], dtype=f)
    l5 = np.asarray(inputs["l5_w"], dtype=f)

    wbig = np.block([[Ur.T, Ui.T], [-Ui.T, Ur.T]]).astype(BF)
    wrt = np.ascontiguousarray(Wr.T).astype(BF)
    wit = np.ascontiguousarray(Wi.T).astype(BF)
    ia = np.zeros((128, 32), f)
    for gg in range(4):
        ia[32 * gg : 32 * gg + 32, :] = np.eye(32, dtype=f)
    ia = ia.astype(BF)
    id64 = np.eye(64, dtype=f).astype(BF)
    w5r = l5[0, :W2]
    w5i = l5[0, W2:]

    l1rT, l1iT = l1r.T, l1i.T   # [768, 3072]
    l2rT, l2iT = l2r.T, l2i.T   # [3072, 3072]
    l3rT, l3iT = l3r.T, l3i.T

    in_maps = []
    for c in range(NC):
        sl = slice(c * BS, (c + 1) * BS)
        fsl = slice(c * FS, (c + 1) * FS)
        xc = x[sl]                                    # [32, 64, 1536]
        xprime = xc.transpose(1, 0, 2).reshape(T * BS, NIN)   # t-major rows
        xt = np.ascontiguousarray(xprime.T).astype(BF)        # [1536, 2048]
        S0 = np.concatenate([h0r[sl], h0i[sl]], axis=0)       # [64, 768]
        s0t = np.ascontiguousarray(
            S0.T.reshape(6, 128, 64).transpose(1, 0, 2)
        ).astype(BF)
        Sn0 = np.concatenate([-h0i[sl], h0r[sl]], axis=0)
        s0nt = np.ascontiguousarray(
            Sn0.T.reshape(6, 128, 64).transpose(1, 0, 2)
        ).astype(BF)
        cw1 = np.concatenate([l1rT[:, fsl], l1iT[:, fsl]], axis=1).astype(BF)
        cw2 = np.concatenate([l2rT[:, fsl], l2iT[:, fsl]], axis=1).astype(BF)
        cw3 = np.concatenate([l3rT[:, fsl], l3iT[:, fsl]], axis=1).astype(BF)
        w5m = np.zeros((128, 6), f)
        for j in range(3):
            w5m[:, j] = w5r[fsl][128 * j : 128 * j + 128]
            w5m[:, 3 + j] = w5i[fsl][128 * j : 128 * j + 128]
        w5m = w5m.astype(BF)
        in_maps.append({
            "xt": xt, "wbig": wbig, "wrt": wrt, "wit": wit,
            "s0t": s0t, "s0nt": s0nt,
            "cw1": cw1, "cw2": cw2, "cw3": cw3, "w5": w5m,
            "ia": ia, "id64": id64,
        })
    return in_maps


def _get_program():
    if "nc" not in _BUILD_CACHE:
        _BUILD_CACHE["nc"] = _build_program()
    return _BUILD_CACHE["nc"]


def kernel(**inputs) -> np.ndarray:
    from concourse.bass_utils import run_bass_kernel_spmd

    nc = _get_program()
    in_maps = _prep_inputs(inputs)
    res = run_bass_kernel_spmd(nc, in_maps, list(range(NC)))
    out = np.asarray(res.results[0]["out"], dtype=np.float32)
    return out

